# revision 23
# baseline (speedup 1.0000x reference)
"""Bahdanau attention Trainium2 kernel.

Math: out = softmax_k(mask(score)) @ values with
  score[b,q,k] = sum_h wv[h] * tanh(Q[b,q,h] + K[b,k,h]),
  Q = queries @ wq, K = keys @ wk.

Key idea: tanh(x) is approximated by a short harmonic sine series
  tanh(x) ~= sum_m alpha_m sin(omega_m x),   omega_m = (m+1)*pi/L
and sin(omega(q+k)) = sin(wq)cos(wk) + cos(wq)sin(wk) factorizes the
[B,Tq,Tk,H] tanh tensor into a handful of dense [Tq,H]x[H,Tk] matmuls on
the PE array. This removes the 134M-element tanh (ACT-bound ~110us/core)
and replaces it with a few us of per-side trig + matmuls.

The scalar engine's Sin is only valid on [-pi, pi] (verified on HW: large
args return garbage), so arguments are range-reduced in 16-bit fixed
point using the HW's round-to-nearest fp32->int32 conversion:
  n = round(x * omega/(2pi) * 65536)     (one tensor_scalar, int32 out)
  v = n & 0xFFFF                         (one tensor_scalar; two's
      complement AND gives the positive residue for negative x free)
  sin: ACT Sin(v, scale=2pi/65536, bias=-pi) = -sin(omega x)
  cos: same chain with +16384 (quarter period) folded into the first op's
      op1=add                            = -cos(omega x)
The minus signs cancel in the sin*cos + cos*sin pairs, so the fold
scalars are simply +alpha_m * wv (per-partition, host-precomputed).

Masked softmax runs without a row max: scores are bounded by ~sum|wv|,
masked entries get -1e6 from a host-built additive tensor so exp
underflows to exactly 0, and rows with valid_len==0 get (m=0, c=0) =>
uniform attention, matching the reference's max-subtracted softmax on an
all(-1e6) row.

Sharding: data-parallel over batch, 2 batches per core on 8 cores.
"""

import math
import sys

import numpy as np

sys.path.insert(0, "/opt/trn_rl_repo")

B, TQ, TK, DIN, H, DV = 16, 128, 256, 64, 256, 256
NCORES = 8
NB = B // NCORES
HB = 2  # h blocks of 128 partitions
NEG = -1000000.0
PI = math.pi
FX = 65536  # fixed-point phase resolution

# (fit range R, sine base half-period L, number of harmonics M)
FIT_LADDER = [
    (5.5, 7.0, 8),
    (7.0, 9.0, 12),
    (9.0, 11.5, 16),
    (12.0, 15.0, 21),
    (16.0, 20.0, 28),
]


def _fit_sine(R, L, M):
    """Least-squares fit tanh(x) ~= sum_m alpha_m sin((m+1) pi x / L) on [-R, R]."""
    x = np.linspace(-R, R, 20001)
    t = np.tanh(x)
    w = 0.05 + np.exp(-0.5 * (x / 0.6) ** 2)
    A = np.stack([np.sin((m + 1) * np.pi * x / L) for m in range(M)], axis=1)
    ATA = (A * w[:, None]).T @ A + 1e-7 * np.eye(M)
    alpha = np.linalg.solve(ATA, (A * w[:, None]).T @ t)
    omega = (np.arange(M) + 1) * np.pi / L
    return alpha.astype(np.float64), omega.astype(np.float64)


def build_program(L, M, alpha=None, exp_shift=0.0):
    """Build the per-core Bass program."""
    import concourse.bacc as bacc
    import concourse.bass as bass
    import concourse.mybir as mybir
    import concourse.tile as tile

    f32 = mybir.dt.float32
    i32 = mybir.dt.int32
    i16 = mybir.dt.int16
    bf16 = mybir.dt.bfloat16
    AF = mybir.ActivationFunctionType
    ALU = mybir.AluOpType

    nc = bacc.Bacc("TRN2", target_bir_lowering=False, debug=False)

    # packed inputs: one DMA each (startup is dominated by per-DMA trigger
    # cost on the HWDGE, ~625ns apiece); q-side packed separately so the
    # q projection can start before the k-side transfer lands
    PQ = H + NB * TQ  # wq | qT
    PK = H + NB * TK  # wk | kT
    P128 = NB * 2 * DV + 128 + HB * M  # vals | identity | fold
    packq_d = nc.dram_tensor("packq", [DIN, PQ], f32, kind="ExternalInput").ap()
    packk_d = nc.dram_tensor("packk", [DIN, PK], f32, kind="ExternalInput").ap()
    pack128_d = nc.dram_tensor("pack128", [128, P128], f32, kind="ExternalInput").ap()
    msmadd_d = nc.dram_tensor("msmadd", [NB, 2, TK], f32, kind="ExternalInput").ap()
    out_d = nc.dram_tensor("out", [NB, TQ, DV], f32, kind="ExternalOutput").ap()

    with tile.TileContext(nc) as tc:
        with (
            tc.tile_pool(name="singles", bufs=1) as singles,
            tc.tile_pool(name="trig", bufs=3) as trig,
            tc.tile_pool(name="soft", bufs=2) as soft,
            tc.tile_pool(name="pproj", bufs=1, space="PSUM") as pproj,
            tc.tile_pool(name="pscore", bufs=2, space="PSUM") as pscore,
            tc.tile_pool(name="ptail", bufs=1, space="PSUM") as ptail,
        ):
            # ---- constants / inputs to SBUF ----
            bias_exp = singles.tile([128, 1], f32)
            nc.vector.memset(bias_exp, -float(exp_shift))

            pkq = singles.tile([DIN, PQ], f32)
            nc.sync.dma_start(out=pkq, in_=packq_d)
            pkk = singles.tile([DIN, PK], f32)
            nc.sync.dma_start(out=pkk, in_=packk_d)
            mm_sb = singles.tile([128, NB, 2, TK], f32)
            nc.sync.dma_start(
                out=mm_sb,
                in_=bass.AP(
                    tensor=msmadd_d.tensor, offset=0, ap=[[0, 128], [1, NB * 2 * TK]]
                ),
            )
            pk128 = singles.tile([128, P128], f32)
            nc.sync.dma_start(out=pk128, in_=pack128_d)

            wq_sb = pkq[:, 0:H]
            qTs = pkq[:, H:].rearrange("p (b x) -> p b x", b=NB)
            wk_sb = pkk[:, 0:H]
            kTs = pkk[:, H:].rearrange("p (b x) -> p b x", b=NB)
            vs = pk128[:, 0 : NB * 2 * DV].rearrange(
                "p (b c v) -> p b c v", b=NB, c=2
            )
            ident_sb = pk128[:, NB * 2 * DV : NB * 2 * DV + 128]
            fold_sb = pk128[:, NB * 2 * DV + 128 :].rearrange(
                "p (hb m) -> p hb m", hb=HB
            )
            ms_sb = mm_sb[:, :, 0, :]
            cs_sb = mm_sb[:, :, 1, :]

            # ---- projections (PE, fp32): [h, hb, b, qi/ki] ----
            # tiny warm-up matmul first: starts the PE p-state ramp early so
            # the projections run at full clock
            warm_ps = pproj.tile([1, 8], f32, tag="warm", name="warm_ps")
            nc.tensor.matmul(
                warm_ps[0:1, 0:1], lhsT=bias_exp, rhs=bias_exp, start=True, stop=True
            )
            qT_ps = pproj.tile([128, HB, NB, TQ], f32)
            for hb in range(HB):
                nc.tensor.matmul(
                    qT_ps[:, hb, :, :].rearrange("p b x -> p (b x)"),
                    lhsT=wq_sb[:, hb * 128 : (hb + 1) * 128],
                    rhs=qTs.rearrange("p b x -> p (b x)"),
                    start=(hb == 0),
                    stop=(hb == HB - 1),
                )
            # kT_ps spans two 2KB psum zero regions (one per hb slice);
            # start/stop must bracket each region's writes separately.
            kT_ps = pproj.tile([128, HB, NB, TK], f32)
            for hb in range(HB):
                for b in range(NB):
                    nc.tensor.matmul(
                        kT_ps[:, hb, b, :],
                        lhsT=wk_sb[:, hb * 128 : (hb + 1) * 128],
                        rhs=kTs[:, b, :],
                        start=(b == 0),
                        stop=(b == NB - 1),
                    )

            qTp = singles.tile([128, HB, NB, TQ], f32)
            nc.vector.tensor_copy(out=qTp, in_=qT_ps)
            kTp = singles.tile([128, HB, NB, TK], f32)
            kTp_copied = [False]

            # ---- score accumulation over m harmonics ----
            scores_ps = [
                pscore.tile([128, TK], f32, tag="scores", name=f"scores{b}")
                for b in range(NB)
            ]
            n_mm = M * 2 * HB  # per batch
            mm_i = [0] * NB
            sin_scale = 2.0 * PI / FX

            def emit_phase(m):
                """DVE int phase converts + ACT Sin for harmonic m.

                Phase tiles hold [hb, sin|cos, b, x] so one ACT Sin sweeps
                both quadratures. The & 0xFFFF is free: ACT reads the LOW
                int16 half of each int32 via a bitcast + stride-2 AP, and the
                SIGNED int16 view puts the phase in [-pi, pi) directly
                (sin arg = v16 * 2pi/65536, no bias), flipping the sign of
                both factors, which cancels in the sin*cos + cos*sin pairs."""
                w16 = float((m + 1) / (2.0 * L) * FX)
                nq = trig.tile([128, HB, 2, NB, TQ], i32, tag="nq", name="nq")
                nc.vector.tensor_scalar(
                    out=nq[:, :, 0], in0=qTp, scalar1=w16, scalar2=None, op0=ALU.mult
                )
                nc.vector.tensor_scalar(
                    out=nq[:, :, 1], in0=qTp, scalar1=w16, scalar2=float(FX // 4),
                    op0=ALU.mult, op1=ALU.add,
                )
                if not kTp_copied[0]:
                    # deferred: q-side m0 ops reach ACT before this big copy
                    nc.vector.tensor_copy(out=kTp, in_=kT_ps)
                    kTp_copied[0] = True
                nk = trig.tile([128, HB, 2, NB, TK], i32, tag="nk", name="nk")
                nc.vector.tensor_scalar(
                    out=nk[:, :, 0], in0=kTp, scalar1=w16, scalar2=None, op0=ALU.mult
                )
                nc.vector.tensor_scalar(
                    out=nk[:, :, 1], in0=kTp, scalar1=w16, scalar2=float(FX // 4),
                    op0=ALU.mult, op1=ALU.add,
                )
                # t[:,hb,0] = sin(w x), t[:,hb,1] = cos(w x)
                tq = trig.tile([128, HB, 2, NB, TQ], bf16, tag="tq", name="tq")
                nc.scalar.activation(
                    out=tq, in_=nq.bitcast(i16)[:, :, :, :, 0::2], func=AF.Sin,
                    bias=0.0, scale=sin_scale,
                )
                tk = trig.tile([128, HB, 2, NB, TK], bf16, tag="tk", name="tk")
                nc.scalar.activation(
                    out=tk, in_=nk.bitcast(i16)[:, :, :, :, 0::2], func=AF.Sin,
                    bias=0.0, scale=sin_scale,
                )
                return tq, tk

            def emit_reduce(m, tq, tk):
                """Fold +alpha_m*wv into the q side (signs cancel in pairs),
                then scores[b] += A.T @ (-cos k) + C.T @ (-sin k)."""
                AC = trig.tile([128, HB, 2, NB, TQ], bf16, tag="AC", name="AC")
                for hb in range(HB):
                    nc.gpsimd.tensor_scalar(
                        out=AC[:, hb], in0=tq[:, hb],
                        scalar1=fold_sb[:, hb, m : m + 1], scalar2=None, op0=ALU.mult,
                    )
                for b in range(NB):
                    for hb in range(HB):
                        nc.tensor.matmul(
                            scores_ps[b],
                            lhsT=AC[:, hb, 0, b, :],
                            rhs=tk[:, hb, 1, b, :],
                            start=(mm_i[b] == 0),
                            stop=(mm_i[b] == n_mm - 1),
                        )
                        mm_i[b] += 1
                        nc.tensor.matmul(
                            scores_ps[b],
                            lhsT=AC[:, hb, 1, b, :],
                            rhs=tk[:, hb, 0, b, :],
                            start=(mm_i[b] == 0),
                            stop=(mm_i[b] == n_mm - 1),
                        )
                        mm_i[b] += 1

            # software-pipeline: folds+matmuls for harmonic m are emitted
            # after phase m+1, so gpsimd's nq(m+1) doesn't queue behind
            # AC(m), which waits on the DVE->ACT chain of m.
            pending = None
            for m in range(M):
                cur = emit_phase(m)
                if pending is not None:
                    emit_reduce(*pending)
                pending = (m, *cur)
            emit_reduce(*pending)

            # ---- masked softmax + attn @ values, per batch ----
            # masking: sm = scores*m + c gives where() semantics (exp of
            # -1e6 underflows to exactly 0). The softmax normalization
            # r=1/sum is folded into the PSUM->SBUF output copy, so the
            # transposes and value matmuls run on unnormalized e.
            out_sb = soft.tile([128, NB, DV], f32, tag="out_sb", name="out_sb")
            for b in range(NB):
                sm = soft.tile([128, TK], f32, tag="sm", name="sm")
                nc.vector.tensor_mul(out=sm, in0=scores_ps[b], in1=ms_sb[:, b, :])
                sm2 = soft.tile([128, TK], f32, tag="sm2", name="sm2")
                nc.vector.tensor_add(out=sm2, in0=sm, in1=cs_sb[:, b, :])
                e = soft.tile([128, TK], f32, tag="e", name="e")
                sums = soft.tile([128, 1], f32, tag="sums", name="sums")
                nc.scalar.activation(
                    out=e, in_=sm2, func=AF.Exp, bias=bias_exp, scale=1.0,
                    accum_out=sums,
                )
                r = soft.tile([128, 1], f32, tag="r", name="r")
                nc.vector.reciprocal(out=r, in_=sums)

                eT_ps = ptail.tile([128, TK], f32, tag="eT_ps", name="eT_ps")
                for c in range(2):
                    nc.tensor.matmul(
                        eT_ps[:, c * 128 : (c + 1) * 128],
                        lhsT=e[:, c * 128 : (c + 1) * 128],
                        rhs=ident_sb,
                        is_transpose=True,
                        start=(c == 0),
                        stop=(c == 1),
                    )
                eT = soft.tile([128, TK], f32, tag="eT", name="eT")
                nc.vector.tensor_copy(out=eT, in_=eT_ps)

                out_ps = ptail.tile([128, DV], f32, tag="out_ps", name="out_ps")
                for c in range(2):
                    nc.tensor.matmul(
                        out_ps,
                        lhsT=eT[:, c * 128 : (c + 1) * 128],
                        rhs=vs[:, b, c, :],
                        start=(c == 0),
                        stop=(c == 1),
                    )
                nc.vector.tensor_scalar(
                    out=out_sb[:, b, :], in0=out_ps, scalar1=r, scalar2=None,
                    op0=ALU.mult,
                )
                nc.sync.dma_start(out=out_d[b], in_=out_sb[:, b, :])

    nc.compile()
    return nc


def prepare_in_maps(queries, keys, values, valid_lens, wq, wk, wv, alpha):
    """Host-side sharding + layout transforms. Returns list of 8 input dicts."""
    M = len(alpha)
    queries = np.ascontiguousarray(queries, dtype=np.float32)
    keys = np.ascontiguousarray(keys, dtype=np.float32)
    values = np.ascontiguousarray(values, dtype=np.float32)
    wq = np.ascontiguousarray(wq, dtype=np.float32)
    wk = np.ascontiguousarray(wk, dtype=np.float32)
    wv = np.asarray(wv, dtype=np.float32)
    valid_lens = np.asarray(valid_lens)

    # fold[p, hb, m] = +alpha_m * wv[hb*128 + p]
    fold = np.empty((128, HB, M), np.float32)
    for hb in range(HB):
        fold[:, hb, :] = np.asarray(alpha, np.float64)[None, :] * wv[
            hb * 128 : (hb + 1) * 128, None
        ]

    ident = np.eye(128, dtype=np.float32)
    ar = np.arange(TK)
    in_maps = []
    for c in range(NCORES):
        bs = slice(c * NB, (c + 1) * NB)
        qT = queries[bs].transpose(2, 0, 1).reshape(DIN, NB * TQ)
        kT = keys[bs].transpose(2, 0, 1).reshape(DIN, NB * TK)
        packq = np.concatenate([wq, qT], axis=1).astype(np.float32)
        packk = np.concatenate([wk, kT], axis=1).astype(np.float32)
        vals = values[bs].reshape(NB, 2, 128, DV).transpose(2, 0, 1, 3)
        pack128 = np.concatenate(
            [vals.reshape(128, NB * 2 * DV), ident,
             fold.reshape(128, HB * len(alpha))], axis=1,
        ).astype(np.float32)
        msmadd = np.empty((NB, 2, TK), np.float32)
        for j, vl in enumerate(valid_lens[bs]):
            vl = int(vl)
            if vl <= 0:
                msmadd[j] = 0.0  # all-masked row -> uniform attention
            else:
                valid = ar < vl
                msmadd[j, 0] = valid.astype(np.float32)
                msmadd[j, 1] = np.where(valid, 0.0, NEG).astype(np.float32)
        in_maps.append(
            {
                "packq": np.ascontiguousarray(packq),
                "packk": np.ascontiguousarray(packk),
                "pack128": np.ascontiguousarray(pack128),
                "msmadd": msmadd,
            }
        )
    return in_maps


def _pick_fit(queries, keys, wq, wk):
    q = queries.reshape(-1, DIN).astype(np.float32) @ wq.astype(np.float32)
    k = keys.reshape(-1, DIN).astype(np.float32) @ wk.astype(np.float32)
    qb = q.reshape(B, TQ, H)
    kb = k.reshape(B, TK, H)
    hi = (qb.max(1) + kb.max(1)).max()
    lo = (qb.min(1) + kb.min(1)).min()
    r_needed = max(abs(hi), abs(lo))
    for R, L, M in FIT_LADDER:
        if R >= r_needed + 0.05:
            return R, L, M
    return FIT_LADDER[-1]


_prog_cache = {}


def kernel(queries, keys, values, valid_lens, wq, wk, wv):
    from concourse import bass_utils

    queries = np.asarray(queries)
    keys = np.asarray(keys)
    values = np.asarray(values)
    valid_lens = np.asarray(valid_lens)
    wq = np.asarray(wq)
    wk = np.asarray(wk)
    wv = np.asarray(wv)

    R, L, M = _pick_fit(queries, keys, wq, wk)
    alpha, omega = _fit_sine(R, L, M)
    # scores are bounded by ~sum|wv| * max|approx tanh|; shift exp if huge
    bound = float(np.abs(wv).sum()) * 1.01
    exp_shift = max(0.0, bound - 60.0)

    key = (R, L, M, round(exp_shift, 3))
    if key not in _prog_cache:
        _prog_cache[key] = build_program(L, M, alpha, exp_shift)
    nc = _prog_cache[key]

    in_maps = prepare_in_maps(queries, keys, values, valid_lens, wq, wk, wv, alpha)
    res = bass_utils.run_bass_kernel_spmd(nc, in_maps, core_ids=list(range(NCORES)))
    out = np.concatenate([r["out"] for r in res.results], axis=0)
    return out.astype(np.float32)


if __name__ == "__main__":
    rng = np.random.default_rng(0)
    inputs = {
        "queries": rng.standard_normal((B, TQ, DIN), dtype=np.float32),
        "keys": rng.standard_normal((B, TK, DIN), dtype=np.float32),
        "values": rng.standard_normal((B, TK, DV), dtype=np.float32),
        "valid_lens": rng.integers(0, TK, size=(B,)).astype(np.int32),
        "wq": (rng.standard_normal((DIN, H), dtype=np.float32) * 0.05),
        "wk": (rng.standard_normal((DIN, H), dtype=np.float32) * 0.05),
        "wv": (rng.standard_normal((H,), dtype=np.float32) * 0.05),
    }
    out = kernel(**inputs)
    print("out", out.shape, out.dtype)
